# revision 15
# baseline (speedup 1.0000x reference)
"""Trainium2 Bass kernel for CMELossAngularProfileMSE_V2.

Strategy (pure data parallel over batch, 8 NeuronCores):
  - Shard B=128 samples -> 16 per core.
  - Host downcasts mask_pred to fp8 e4m3 (values in [0,1), RNE rounding:
    quantization noise on the per-(b,theta) radial mean A is ~4e-4 after
    averaging 2048 samples; final loss rel-err ~1e-4, far inside the
    2e-2 gate) -> 4x less HBM traffic than fp32.
  - Per core, per sample: DMA a [128, 5760B] fp8 tile (partition p holds
    r in [16p, 16p+16), free dim = q-major 16*360 contiguous).
  - The whole radial reduction runs on the Tensor engine as DoubleRow
    fp8 matmuls (2 fp8 MACs per PE cell per cycle): per sample, 8
    matmuls of rhs [128, 2(pair, stride 720B), 360] against a one-hot
    ones weight [128, 2, 16] (column b), accumulating over the pair
    dim, the partition dim, the 8 chunk-matmuls, and all 16 samples
    into a single PSUM tile [16, 360] holding raw radial sums S[b,th].
    DVE stays idle for the bulk (its fp8 tensor_tensor is 1x mode and
    would be the bottleneck).
  - Host precomputes T' = R*T and w' = w/R^2 (exact power-of-two
    scalings of the Gaussian target / distance weight derived from
    theta_min/theta_max), so the device epilogue is just
    sum_theta((S - T')^2 * w') per sample -> out [16, 1], on DVE.
  - Host: loss = sum(all per-sample sums) / (360 * 128).
"""
import numpy as np
import ml_dtypes

import concourse.bacc as bacc
import concourse.tile as tile
from concourse import mybir
from concourse.bass_utils import run_bass_kernel_spmd

F32 = mybir.dt.float32
FP8 = mybir.dt.float8e4

N_CORES = 8
B = 128            # full batch
BS = B // N_CORES  # samples per core (16)
R = 2048
TH = 360
Q = 16             # r-slices per partition (2048 = 128 * 16)
SIGMA = 10.0
ALPHA_WEIGHT = 2.0
LAMBDA_ANG = 1.0


def _build_nc():
    nc = bacc.Bacc("TRN2", target_bir_lowering=False, debug=False)
    # q = 4*c + 2*pair + j: a plain reshape of the q-major layout gives
    # DoubleRow pair components 2 q-slices (720 B) apart -- 16B-aligned
    # strides for the PE weight/moving APs, no host shuffle needed.
    x = nc.dram_tensor("x", [BS, 128, 4, 2, 2, TH], FP8, kind="ExternalInput").ap()
    tw = nc.dram_tensor("tw", [2, BS, TH], F32, kind="ExternalInput").ap()
    out = nc.dram_tensor("out", [BS, 1], F32, kind="ExternalOutput").ap()

    from contextlib import ExitStack
    with tile.TileContext(nc) as tc, ExitStack() as ctx:
        consts = ctx.enter_context(tc.tile_pool(name="consts", bufs=1))
        # all 16 sample tiles stay resident (90KB/partition): every bulk
        # DMA dispatches immediately with no pool-recycle WAR waits, so
        # the stream runs at the pure-DMA rate with compute chasing it.
        inp = ctx.enter_context(tc.tile_pool(name="inp", bufs=BS))
        psum = ctx.enter_context(tc.tile_pool(name="psum", bufs=1, space="PSUM"))
        small = ctx.enter_context(tc.tile_pool(name="small", bufs=1))

        HB = BS // 2  # psum group size (8)

        # one-hot DoubleRow weight stack: W[p, b, i, m] = 1 iff m == b%8
        # (sample b's matmuls use W[:, b] = ones in column b%8 across
        # both pair halves; samples 0-7 and 8-15 accumulate into two
        # separate PSUM groups so the first epilogue hides under the
        # second half's stream). gpsimd memsets stay off the DMA rings.
        # pair-halves padded to 16 cols: dual-fp8 LDWEIGHTS requires the
        # pair step to be a multiple of 16 bytes (s3_lw restriction);
        # matmuls slice the first HB columns of each half.
        W = consts.tile([128, BS, 2, 2 * HB], FP8)
        nc.gpsimd.memset(W[:], 0.0)
        for b in range(BS):
            nc.gpsimd.memset(W[:, b, :, b % HB : b % HB + 1], 1.0)

        # tw holds T' = R*T and w' = w/R^2 (exact power-of-two scalings),
        # so the raw PSUM sums S feed the epilogue directly. Issued on
        # the scalar HWDGE ring: the sync ring's head-of-queue slot
        # stays with the bulk stream.
        # per-group epilogue tiles (DVE ops need 32-aligned partition
        # bases, so group B gets its own tiles at partition 0)
        t8w8_a = small.tile([HB, 2, TH], F32)
        t8w8_b = small.tile([HB, 2, TH], F32)
        tws = [t8w8_a, t8w8_b]
        # SWDGE (gpsimd) ring: keeps both HWDGE rings free for the bulk
        # stream; only needed by the epilogues much later.
        nc.gpsimd.dma_start(t8w8_a[:], tw[:, :HB].rearrange("two b t -> b two t"))
        nc.gpsimd.dma_start(t8w8_b[:], tw[:, HB:].rearrange("two b t -> b two t"))

        BF16 = mybir.dt.bfloat16
        d8_a = small.tile([HB, TH], BF16)
        d8_b = small.tile([HB, TH], BF16)
        sq8_a = small.tile([HB, TH], BF16)
        sq8_b = small.tile([HB, TH], BF16)
        w8_a = small.tile([HB, TH], BF16)
        w8_b = small.tile([HB, TH], BF16)
        sqw8_a = small.tile([HB, TH], BF16)
        sqw8_b = small.tile([HB, TH], BF16)
        red_a = small.tile([HB, 1], F32)
        red_b = small.tile([HB, 1], F32)
        d8s, sq8s, w8s, sqw8s, reds = [d8_a, d8_b], [sq8_a, sq8_b], \
            [w8_a, w8_b], [sqw8_a, sqw8_b], [red_a, red_b]

        # w' downcast to bf16 up front (DVE is idle mid-stream) so the
        # serial tail stages run at the 16-bit 2x DVE rate; d is O(R),
        # so bf16's 0.4% rounding is noise vs the 2e-2 gate.
        nc.vector.tensor_copy(w8_a[:], t8w8_a[:, 1, :])
        nc.vector.tensor_copy(w8_b[:], t8w8_b[:, 1, :])

        def epilogue(g):
            ps, twg = pss[g], tws[g]
            d8, sq8, sqw8, red = d8s[g], sq8s[g], sqw8s[g], reds[g]
            nc.vector.scalar_tensor_tensor(
                d8[:], ps[:], 1.0, twg[:, 0, :],
                op0=mybir.AluOpType.mult, op1=mybir.AluOpType.subtract,
            )
            nc.vector.scalar_tensor_tensor(
                sq8[:], d8[:], 1.0, d8[:],
                op0=mybir.AluOpType.mult, op1=mybir.AluOpType.mult,
            )
            nc.vector.scalar_tensor_tensor(
                sqw8[:], sq8[:], 1.0, w8s[g][:],
                op0=mybir.AluOpType.mult, op1=mybir.AluOpType.mult,
                accum_out=red[:],
            )
            nc.sync.dma_start(out[g * HB : (g + 1) * HB], red[:])

        ps_a = psum.tile([HB, TH], F32)
        ps_b = psum.tile([HB, TH], F32)
        pss = [ps_a, ps_b]
        for b in range(BS):
            xt = inp.tile([128, 4, 2, 2, TH], FP8)
            # one full 720KB transfer per sample (matches the measured
            # pure-DMA-rate structure; the two HWDGE rings alternate so
            # both dispatch concurrently); only the last sample streams
            # in quarters so the final matmuls trail just 1440B lines.
            ring = nc.sync if b % 2 == 0 else nc.scalar
            n_chunks = 4 if b == BS - 1 else 1
            step = 4 // n_chunks
            for c0 in range(0, 4, step):
                ring.dma_start(
                    xt[:, c0 : c0 + step], x[b][:, c0 : c0 + step]
                )
            g, ps = b // HB, pss[b // HB]
            for c in range(4):
                for j in range(2):
                    nc.tensor.matmul(
                        ps[:], W[:, b, :, :HB], xt[:, c, :, j, :],
                        start=(b % HB == 0 and c == 0 and j == 0),
                        stop=(b % HB == HB - 1 and c == 3 and j == 1),
                        perf_mode=mybir.MatmulPerfMode.DoubleRow,
                    )
            if b % HB == HB - 1:
                epilogue(g)
    nc.compile()
    return nc


def _target_and_weight(theta_min: np.ndarray, theta_max: np.ndarray):
    """Gaussian soft target T and distance weight w, [B, TH] float32 each.

    Mirrors the reference formulas (computed in float64, cast to float32;
    differences vs the f32 jax pipeline are O(1 ulp))."""
    theta = np.arange(TH, dtype=np.float64)[None, None, :]      # [1, 1, TH]
    tmin = theta_min.astype(np.float64)[:, :, None]             # [B, K, 1]
    tmax = theta_max.astype(np.float64)[:, :, None]

    center_wrap = np.mod(0.5 * (tmin + tmax + 360.0), 360.0)
    center_t = np.where(tmin <= tmax, 0.5 * (tmin + tmax), center_wrap)
    d = np.abs(theta - center_t)
    dist_t = np.minimum(d, 360.0 - d)                           # [B, K, TH]
    T = np.clip(np.exp(-0.5 * (dist_t / SIGMA) ** 2).sum(axis=1), 0.0, 1.0)

    center_w = (tmin + np.mod(tmax - tmin, 360.0)) / 2.0
    dw = np.abs(theta - center_w)
    dist_w = np.minimum(dw, 360.0 - dw)
    w = 1.0 + ALPHA_WEIGHT * (dist_w.max(axis=1) / 180.0)       # [B, TH]

    # Feed the device T' = R*T and w' = w/R^2 (both exact scalings by
    # powers of two) so it can use the raw radial sums S instead of the
    # mean A = S/R:  ((S - R*T)^2 * w/R^2) == ((A - T)^2 * w).
    Tp = (T * np.float32(R)).astype(np.float32)
    wp = (w / np.float32(R) ** 2).astype(np.float32)
    return Tp, wp


_NC_CACHE = None


def _get_nc():
    global _NC_CACHE
    if _NC_CACHE is None:
        _NC_CACHE = _build_nc()
    return _NC_CACHE


def _run(mask_pred, theta_min, theta_max, trace=False, trace_kwargs=None,
         trace_cores=None):
    mask_pred = np.asarray(mask_pred, dtype=np.float32)
    theta_min = np.asarray(theta_min)
    theta_max = np.asarray(theta_max)
    T, w = _target_and_weight(theta_min, theta_max)

    # One fp8 conversion pass over the full batch; per-core tensors are
    # then zero-copy reshapes of contiguous slices.
    x8 = np.ascontiguousarray(mask_pred[:, 0]).astype(ml_dtypes.float8_e4m3)

    in_maps = []
    for i in range(N_CORES):
        sl = slice(i * BS, (i + 1) * BS)
        x_core = x8[sl].reshape(BS, 128, 4, 2, 2, TH)
        tw_core = np.stack([T[sl], w[sl]])
        in_maps.append({"x": x_core, "tw": tw_core})

    kwargs = {}
    if trace:
        kwargs["trace"] = True
        if trace_kwargs:
            kwargs["trace_kwargs"] = trace_kwargs
        if trace_cores is not None:
            kwargs["trace_cores"] = trace_cores
    res = run_bass_kernel_spmd(_get_nc(), in_maps, core_ids=list(range(N_CORES)),
                               **kwargs)
    per_sample = np.concatenate(
        [res.results[i]["out"][:, 0] for i in range(N_CORES)]
    )
    total = per_sample.astype(np.float64).sum() / (TH * B)
    return np.float32(LAMBDA_ANG * total), res


def kernel(mask_pred: np.ndarray, theta_min: np.ndarray,
           theta_max: np.ndarray) -> np.ndarray:
    loss, _ = _run(mask_pred, theta_min, theta_max)
    return np.asarray(loss, dtype=np.float32)


# revision 16
# speedup vs baseline: 1.0133x; 1.0133x over previous
"""Trainium2 Bass kernel for CMELossAngularProfileMSE_V2.

Strategy (pure data parallel over batch, 8 NeuronCores):
  - Shard B=128 samples -> 16 per core.
  - Host downcasts mask_pred to fp8 e4m3 (values in [0,1), RNE rounding:
    quantization noise on the per-(b,theta) radial mean A is ~4e-4 after
    averaging 2048 samples; final loss rel-err ~1e-4, far inside the
    2e-2 gate) -> 4x less HBM traffic than fp32.
  - Per core, per sample: DMA a [128, 5760B] fp8 tile (partition p holds
    r in [16p, 16p+16), free dim = q-major 16*360 contiguous).
  - The whole radial reduction runs on the Tensor engine as DoubleRow
    fp8 matmuls (2 fp8 MACs per PE cell per cycle, 152ns/MM warm): per
    sample, 8 matmuls of rhs [128, 2(pair, stride 720B), 360] against
    a one-hot ones weight [128, 2, 8] (column b%8, pair halves padded
    to 16 cols for the s3_lw 16B-step rule), accumulating pair dim +
    partition dim + 8 chunk-matmuls + 8 samples into one of two PSUM
    group tiles [8, 360] holding raw radial sums S[b,th]. Two groups
    (samples 0-7, 8-15) let the first epilogue hide under the second
    half's stream. DVE stays idle for the bulk (its fp8 tensor_tensor
    is 1x mode and would be the bottleneck).
  - Host precomputes T' = R*T and w' = w/R^2 (exact power-of-two
    scalings of the Gaussian target / distance weight derived from
    theta_min/theta_max), so the device epilogue is just
    sum_theta((S - T')^2 * w') per sample -> out [16, 1], on DVE.
  - Host: loss = sum(all per-sample sums) / (360 * 128).
"""
import numpy as np
import ml_dtypes

import concourse.bacc as bacc
import concourse.tile as tile
from concourse import mybir
from concourse.bass_utils import run_bass_kernel_spmd

F32 = mybir.dt.float32
FP8 = mybir.dt.float8e4

N_CORES = 8
B = 128            # full batch
BS = B // N_CORES  # samples per core (16)
R = 2048
TH = 360
Q = 16             # r-slices per partition (2048 = 128 * 16)
SIGMA = 10.0
ALPHA_WEIGHT = 2.0
LAMBDA_ANG = 1.0


def _build_nc():
    nc = bacc.Bacc("TRN2", target_bir_lowering=False, debug=False)
    # q = 4*c + 2*pair + j: a plain reshape of the q-major layout gives
    # DoubleRow pair components 2 q-slices (720 B) apart -- 16B-aligned
    # strides for the PE weight/moving APs, no host shuffle needed.
    x = nc.dram_tensor("x", [BS, 128, 4, 2, 2, TH], FP8, kind="ExternalInput").ap()
    tw = nc.dram_tensor("tw", [2, BS, TH], F32, kind="ExternalInput").ap()
    out = nc.dram_tensor("out", [BS, 1], F32, kind="ExternalOutput").ap()

    from contextlib import ExitStack
    with tile.TileContext(nc) as tc, ExitStack() as ctx:
        consts = ctx.enter_context(tc.tile_pool(name="consts", bufs=1))
        # all 16 sample tiles stay resident (90KB/partition): every bulk
        # DMA dispatches immediately with no pool-recycle WAR waits, so
        # the stream runs at the pure-DMA rate with compute chasing it.
        inp = ctx.enter_context(tc.tile_pool(name="inp", bufs=BS))
        psum = ctx.enter_context(tc.tile_pool(name="psum", bufs=1, space="PSUM"))
        small = ctx.enter_context(tc.tile_pool(name="small", bufs=1))

        HB = BS // 2  # psum group size (8)

        # one-hot DoubleRow weight stack: W[p, b, i, m] = 1 iff m == b%8
        # (sample b's matmuls use W[:, b] = ones in column b%8 across
        # both pair halves; samples 0-7 and 8-15 accumulate into two
        # separate PSUM groups so the first epilogue hides under the
        # second half's stream). gpsimd memsets stay off the DMA rings.
        # pair-halves padded to 16 cols: dual-fp8 LDWEIGHTS requires the
        # pair step to be a multiple of 16 bytes (s3_lw restriction);
        # matmuls slice the first HB columns of each half.
        W = consts.tile([128, BS, 2, 2 * HB], FP8)
        nc.gpsimd.memset(W[:], 0.0)
        for b in range(BS):
            nc.gpsimd.memset(W[:, b, :, b % HB : b % HB + 1], 1.0)

        # tw holds T' = R*T and w' = w/R^2 (exact power-of-two scalings),
        # so the raw PSUM sums S feed the epilogue directly. Issued on
        # the scalar HWDGE ring: the sync ring's head-of-queue slot
        # stays with the bulk stream.
        # per-group epilogue tiles (DVE ops need 32-aligned partition
        # bases, so group B gets its own tiles at partition 0)
        t8w8_a = small.tile([HB, 2, TH], F32)
        t8w8_b = small.tile([HB, 2, TH], F32)
        tws = [t8w8_a, t8w8_b]
        # SWDGE (gpsimd) ring: keeps both HWDGE rings free for the bulk
        # stream; only needed by the epilogues much later.
        nc.gpsimd.dma_start(t8w8_a[:], tw[:, :HB].rearrange("two b t -> b two t"))
        nc.gpsimd.dma_start(t8w8_b[:], tw[:, HB:].rearrange("two b t -> b two t"))

        BF16 = mybir.dt.bfloat16
        d8_a = small.tile([HB, TH], BF16)
        d8_b = small.tile([HB, TH], BF16)
        sq8_a = small.tile([HB, TH], BF16)
        sq8_b = small.tile([HB, TH], BF16)
        w8_a = small.tile([HB, TH], BF16)
        w8_b = small.tile([HB, TH], BF16)
        sqw8_a = small.tile([HB, TH], BF16)
        sqw8_b = small.tile([HB, TH], BF16)
        red_a = small.tile([HB, 1], F32)
        red_b = small.tile([HB, 1], F32)
        d8s, sq8s, w8s, sqw8s, reds = [d8_a, d8_b], [sq8_a, sq8_b], \
            [w8_a, w8_b], [sqw8_a, sqw8_b], [red_a, red_b]

        # w' downcast to bf16 up front (DVE is idle mid-stream) so the
        # serial tail stages run at the 16-bit 2x DVE rate; d is O(R),
        # so bf16's 0.4% rounding is noise vs the 2e-2 gate.
        nc.vector.tensor_copy(w8_a[:], t8w8_a[:, 1, :])
        nc.vector.tensor_copy(w8_b[:], t8w8_b[:, 1, :])

        def epilogue(g):
            ps, twg = pss[g], tws[g]
            d8, sq8, sqw8, red = d8s[g], sq8s[g], sqw8s[g], reds[g]
            nc.vector.scalar_tensor_tensor(
                d8[:], ps[:], 1.0, twg[:, 0, :],
                op0=mybir.AluOpType.mult, op1=mybir.AluOpType.subtract,
            )
            nc.vector.scalar_tensor_tensor(
                sq8[:], d8[:], 1.0, d8[:],
                op0=mybir.AluOpType.mult, op1=mybir.AluOpType.mult,
            )
            nc.vector.scalar_tensor_tensor(
                sqw8[:], sq8[:], 1.0, w8s[g][:],
                op0=mybir.AluOpType.mult, op1=mybir.AluOpType.mult,
                accum_out=red[:],
            )
            nc.sync.dma_start(out[g * HB : (g + 1) * HB], red[:])

        ps_a = psum.tile([HB, TH], F32)
        ps_b = psum.tile([HB, TH], F32)
        pss = [ps_a, ps_b]
        for b in range(BS):
            xt = inp.tile([128, 4, 2, 2, TH], FP8)
            # one full 720KB transfer per sample (matches the measured
            # pure-DMA-rate structure; the two HWDGE rings alternate so
            # both dispatch concurrently); only the last sample streams
            # in quarters so the final matmuls trail just 1440B lines.
            ring = nc.sync if b % 2 == 0 else nc.scalar
            n_chunks = 4 if b == BS - 1 else 1
            step = 4 // n_chunks
            for c0 in range(0, 4, step):
                ring.dma_start(
                    xt[:, c0 : c0 + step], x[b][:, c0 : c0 + step]
                )
            g, ps = b // HB, pss[b // HB]
            for c in range(4):
                for j in range(2):
                    nc.tensor.matmul(
                        ps[:], W[:, b, :, :HB], xt[:, c, :, j, :],
                        start=(b % HB == 0 and c == 0 and j == 0),
                        stop=(b % HB == HB - 1 and c == 3 and j == 1),
                        perf_mode=mybir.MatmulPerfMode.DoubleRow,
                    )
            if b % HB == HB - 1:
                epilogue(g)
    nc.compile()
    return nc


def _target_and_weight(theta_min: np.ndarray, theta_max: np.ndarray):
    """Gaussian soft target T and distance weight w, [B, TH] float32 each.

    Mirrors the reference formulas (computed in float64, cast to float32;
    differences vs the f32 jax pipeline are O(1 ulp))."""
    theta = np.arange(TH, dtype=np.float64)[None, None, :]      # [1, 1, TH]
    tmin = theta_min.astype(np.float64)[:, :, None]             # [B, K, 1]
    tmax = theta_max.astype(np.float64)[:, :, None]

    center_wrap = np.mod(0.5 * (tmin + tmax + 360.0), 360.0)
    center_t = np.where(tmin <= tmax, 0.5 * (tmin + tmax), center_wrap)
    d = np.abs(theta - center_t)
    dist_t = np.minimum(d, 360.0 - d)                           # [B, K, TH]
    T = np.clip(np.exp(-0.5 * (dist_t / SIGMA) ** 2).sum(axis=1), 0.0, 1.0)

    center_w = (tmin + np.mod(tmax - tmin, 360.0)) / 2.0
    dw = np.abs(theta - center_w)
    dist_w = np.minimum(dw, 360.0 - dw)
    w = 1.0 + ALPHA_WEIGHT * (dist_w.max(axis=1) / 180.0)       # [B, TH]

    # Feed the device T' = R*T and w' = w/R^2 (both exact scalings by
    # powers of two) so it can use the raw radial sums S instead of the
    # mean A = S/R:  ((S - R*T)^2 * w/R^2) == ((A - T)^2 * w).
    Tp = (T * np.float32(R)).astype(np.float32)
    wp = (w / np.float32(R) ** 2).astype(np.float32)
    return Tp, wp


_NC_CACHE = None


def _get_nc():
    global _NC_CACHE
    if _NC_CACHE is None:
        _NC_CACHE = _build_nc()
    return _NC_CACHE


def _run(mask_pred, theta_min, theta_max, trace=False, trace_kwargs=None,
         trace_cores=None):
    mask_pred = np.asarray(mask_pred, dtype=np.float32)
    theta_min = np.asarray(theta_min)
    theta_max = np.asarray(theta_max)
    T, w = _target_and_weight(theta_min, theta_max)

    # One fp8 conversion pass over the full batch; per-core tensors are
    # then zero-copy reshapes of contiguous slices.
    x8 = np.ascontiguousarray(mask_pred[:, 0]).astype(ml_dtypes.float8_e4m3)

    in_maps = []
    for i in range(N_CORES):
        sl = slice(i * BS, (i + 1) * BS)
        x_core = x8[sl].reshape(BS, 128, 4, 2, 2, TH)
        tw_core = np.stack([T[sl], w[sl]])
        in_maps.append({"x": x_core, "tw": tw_core})

    kwargs = {}
    if trace:
        kwargs["trace"] = True
        if trace_kwargs:
            kwargs["trace_kwargs"] = trace_kwargs
        if trace_cores is not None:
            kwargs["trace_cores"] = trace_cores
    res = run_bass_kernel_spmd(_get_nc(), in_maps, core_ids=list(range(N_CORES)),
                               **kwargs)
    per_sample = np.concatenate(
        [res.results[i]["out"][:, 0] for i in range(N_CORES)]
    )
    total = per_sample.astype(np.float64).sum() / (TH * B)
    return np.float32(LAMBDA_ANG * total), res


def kernel(mask_pred: np.ndarray, theta_min: np.ndarray,
           theta_max: np.ndarray) -> np.ndarray:
    loss, _ = _run(mask_pred, theta_min, theta_max)
    return np.asarray(loss, dtype=np.float32)


# revision 17
# speedup vs baseline: 1.0691x; 1.0551x over previous
"""Trainium2 Bass kernel for CMELossAngularProfileMSE_V2.

Strategy (pure data parallel over batch, 8 NeuronCores):
  - Shard B=128 samples -> 16 per core.
  - Host downcasts mask_pred to fp8 e4m3 (values in [0,1), RNE rounding:
    quantization noise on the per-(b,theta) radial mean A is ~4e-4 after
    averaging 2048 samples; final loss rel-err ~1e-4, far inside the
    2e-2 gate) -> 4x less HBM traffic than fp32.
  - Per core, per sample: DMA a [128, 5760B] fp8 tile (partition p holds
    r in [16p, 16p+16), free dim = q-major 16*360 contiguous).
  - The whole radial reduction runs on the Tensor engine as DoubleRow
    fp8 matmuls (2 fp8 MACs per PE cell per cycle, 152ns/MM warm): per
    sample, 8 matmuls of rhs [128, 2(pair, stride 720B), 360] against
    a one-hot ones weight [128, 2, 8] (column b%8, pair halves padded
    to 16 cols for the s3_lw 16B-step rule), accumulating pair dim +
    partition dim + 8 chunk-matmuls + 8 samples into one of two PSUM
    group tiles [8, 360] holding raw radial sums S[b,th]. Two groups
    (samples 0-7, 8-15) let the first epilogue hide under the second
    half's stream. DVE stays idle for the bulk (its fp8 tensor_tensor
    is 1x mode and would be the bottleneck).
  - Host precomputes T' = R*T and w' = w/R^2 (exact power-of-two
    scalings of the Gaussian target / distance weight derived from
    theta_min/theta_max), so the device epilogue is just
    sum_theta((S - T')^2 * w') per sample -> out [16, 1], on DVE.
  - Host: loss = sum(all per-sample sums) / (360 * 128).
"""
import numpy as np
import ml_dtypes

import concourse.bacc as bacc
import concourse.tile as tile
from concourse import mybir
from concourse.bass_utils import run_bass_kernel_spmd

F32 = mybir.dt.float32
FP8 = mybir.dt.float8e4

N_CORES = 8
B = 128            # full batch
BS = B // N_CORES  # samples per core (16)
R = 2048
TH = 360
Q = 16             # r-slices per partition (2048 = 128 * 16)
SIGMA = 10.0
ALPHA_WEIGHT = 2.0
LAMBDA_ANG = 1.0


def _build_nc():
    nc = bacc.Bacc("TRN2", target_bir_lowering=False, debug=False)
    # q = 4*c + 2*pair + j: a plain reshape of the q-major layout gives
    # DoubleRow pair components 2 q-slices (720 B) apart -- 16B-aligned
    # strides for the PE weight/moving APs, no host shuffle needed.
    x = nc.dram_tensor("x", [BS, 128, 4, 2, 2, TH], FP8, kind="ExternalInput").ap()
    tw = nc.dram_tensor("tw", [2, BS, TH], F32, kind="ExternalInput").ap()
    out = nc.dram_tensor("out", [BS, 1], F32, kind="ExternalOutput").ap()

    from contextlib import ExitStack
    with tile.TileContext(nc) as tc, ExitStack() as ctx:
        consts = ctx.enter_context(tc.tile_pool(name="consts", bufs=1))
        # all 16 sample tiles stay resident (90KB/partition): every bulk
        # DMA dispatches immediately with no pool-recycle WAR waits, so
        # the stream runs at the pure-DMA rate with compute chasing it.
        inp = ctx.enter_context(tc.tile_pool(name="inp", bufs=BS))
        psum = ctx.enter_context(tc.tile_pool(name="psum", bufs=1, space="PSUM"))
        small = ctx.enter_context(tc.tile_pool(name="small", bufs=1))

        HB = BS // 2  # psum group size (8)

        # one-hot DoubleRow weight stack: W[p, b, i, m] = 1 iff m == b%8
        # (sample b's matmuls use W[:, b] = ones in column b%8 across
        # both pair halves; samples 0-7 and 8-15 accumulate into two
        # separate PSUM groups so the first epilogue hides under the
        # second half's stream). gpsimd memsets stay off the DMA rings.
        # pair-halves padded to 16 cols: dual-fp8 LDWEIGHTS requires the
        # pair step to be a multiple of 16 bytes (s3_lw restriction);
        # matmuls slice the first HB columns of each half.
        W = consts.tile([128, BS, 2, 2 * HB], FP8)
        nc.gpsimd.memset(W[:], 0.0)
        for b in range(BS):
            nc.gpsimd.memset(W[:, b, :, b % HB : b % HB + 1], 1.0)

        # tw holds T' = R*T and w' = w/R^2 (exact power-of-two scalings),
        # so the raw PSUM sums S feed the epilogue directly. Issued on
        # the scalar HWDGE ring: the sync ring's head-of-queue slot
        # stays with the bulk stream.
        # per-group epilogue tiles (DVE ops need 32-aligned partition
        # bases, so group B gets its own tiles at partition 0)
        t8w8_a = small.tile([HB, 2, TH], F32)
        t8w8_b = small.tile([HB, 2, TH], F32)
        tws = [t8w8_a, t8w8_b]
        # SWDGE (gpsimd) ring: keeps both HWDGE rings free for the bulk
        # stream; only needed by the epilogues much later.
        nc.gpsimd.dma_start(t8w8_a[:], tw[:, :HB].rearrange("two b t -> b two t"))
        nc.gpsimd.dma_start(t8w8_b[:], tw[:, HB:].rearrange("two b t -> b two t"))

        BF16 = mybir.dt.bfloat16
        d8_a = small.tile([HB, TH], BF16)
        d8_b = small.tile([HB, TH], BF16)
        sq8_a = small.tile([HB, TH], BF16)
        sq8_b = small.tile([HB, TH], BF16)
        w8_a = small.tile([HB, TH], BF16)
        w8_b = small.tile([HB, TH], BF16)
        sqw8_a = small.tile([HB, TH], BF16)
        sqw8_b = small.tile([HB, TH], BF16)
        red_a = small.tile([HB, 1], F32)
        red_b = small.tile([HB, 1], F32)
        d8s, sq8s, w8s, sqw8s, reds = [d8_a, d8_b], [sq8_a, sq8_b], \
            [w8_a, w8_b], [sqw8_a, sqw8_b], [red_a, red_b]

        # w' downcast to bf16 up front (DVE is idle mid-stream) so the
        # serial tail stages run at the 16-bit 2x DVE rate; d is O(R),
        # so bf16's 0.4% rounding is noise vs the 2e-2 gate.
        nc.vector.tensor_copy(w8_a[:], t8w8_a[:, 1, :])
        nc.vector.tensor_copy(w8_b[:], t8w8_b[:, 1, :])

        def epilogue(g):
            ps, twg = pss[g], tws[g]
            d8, sq8, sqw8, red = d8s[g], sq8s[g], sqw8s[g], reds[g]
            nc.vector.scalar_tensor_tensor(
                d8[:], ps[:], 1.0, twg[:, 0, :],
                op0=mybir.AluOpType.mult, op1=mybir.AluOpType.subtract,
            )
            nc.vector.scalar_tensor_tensor(
                sq8[:], d8[:], 1.0, d8[:],
                op0=mybir.AluOpType.mult, op1=mybir.AluOpType.mult,
            )
            nc.vector.scalar_tensor_tensor(
                sqw8[:], sq8[:], 1.0, w8s[g][:],
                op0=mybir.AluOpType.mult, op1=mybir.AluOpType.mult,
                accum_out=red[:],
            )
            nc.sync.dma_start(out[g * HB : (g + 1) * HB], red[:])

        ps_a = psum.tile([HB, TH], F32)
        ps_b = psum.tile([HB, TH], F32)
        pss = [ps_a, ps_b]
        for b in range(BS):
            xt = inp.tile([128, 4, 2, 2, TH], FP8)
            # one full 720KB transfer per sample (matches the measured
            # pure-DMA-rate structure; the two HWDGE rings alternate so
            # both dispatch concurrently); only the last sample streams
            # in quarters so the final matmuls trail just 1440B lines.
            ring = nc.sync if b % 2 == 0 else nc.scalar
            n_chunks = 4 if b == BS - 1 else 1
            step = 4 // n_chunks
            for c0 in range(0, 4, step):
                # last sample: alternate rings per quarter so the
                # completion sems (HBM receipt ~1us) fire in parallel
                if n_chunks == 4:
                    ring = nc.scalar if c0 % 2 == 0 else nc.sync
                ring.dma_start(
                    xt[:, c0 : c0 + step], x[b][:, c0 : c0 + step]
                )
            g, ps = b // HB, pss[b // HB]
            for c in range(4):
                for j in range(2):
                    nc.tensor.matmul(
                        ps[:], W[:, b, :, :HB], xt[:, c, :, j, :],
                        start=(b % HB == 0 and c == 0 and j == 0),
                        stop=(b % HB == HB - 1 and c == 3 and j == 1),
                        perf_mode=mybir.MatmulPerfMode.DoubleRow,
                    )
            if b % HB == HB - 1:
                epilogue(g)
    nc.compile()
    return nc


def _target_and_weight(theta_min: np.ndarray, theta_max: np.ndarray):
    """Gaussian soft target T and distance weight w, [B, TH] float32 each.

    Mirrors the reference formulas (computed in float64, cast to float32;
    differences vs the f32 jax pipeline are O(1 ulp))."""
    theta = np.arange(TH, dtype=np.float64)[None, None, :]      # [1, 1, TH]
    tmin = theta_min.astype(np.float64)[:, :, None]             # [B, K, 1]
    tmax = theta_max.astype(np.float64)[:, :, None]

    center_wrap = np.mod(0.5 * (tmin + tmax + 360.0), 360.0)
    center_t = np.where(tmin <= tmax, 0.5 * (tmin + tmax), center_wrap)
    d = np.abs(theta - center_t)
    dist_t = np.minimum(d, 360.0 - d)                           # [B, K, TH]
    T = np.clip(np.exp(-0.5 * (dist_t / SIGMA) ** 2).sum(axis=1), 0.0, 1.0)

    center_w = (tmin + np.mod(tmax - tmin, 360.0)) / 2.0
    dw = np.abs(theta - center_w)
    dist_w = np.minimum(dw, 360.0 - dw)
    w = 1.0 + ALPHA_WEIGHT * (dist_w.max(axis=1) / 180.0)       # [B, TH]

    # Feed the device T' = R*T and w' = w/R^2 (both exact scalings by
    # powers of two) so it can use the raw radial sums S instead of the
    # mean A = S/R:  ((S - R*T)^2 * w/R^2) == ((A - T)^2 * w).
    Tp = (T * np.float32(R)).astype(np.float32)
    wp = (w / np.float32(R) ** 2).astype(np.float32)
    return Tp, wp


_NC_CACHE = None


def _get_nc():
    global _NC_CACHE
    if _NC_CACHE is None:
        _NC_CACHE = _build_nc()
    return _NC_CACHE


def _run(mask_pred, theta_min, theta_max, trace=False, trace_kwargs=None,
         trace_cores=None):
    mask_pred = np.asarray(mask_pred, dtype=np.float32)
    theta_min = np.asarray(theta_min)
    theta_max = np.asarray(theta_max)
    T, w = _target_and_weight(theta_min, theta_max)

    # One fp8 conversion pass over the full batch; per-core tensors are
    # then zero-copy reshapes of contiguous slices.
    x8 = np.ascontiguousarray(mask_pred[:, 0]).astype(ml_dtypes.float8_e4m3)

    in_maps = []
    for i in range(N_CORES):
        sl = slice(i * BS, (i + 1) * BS)
        x_core = x8[sl].reshape(BS, 128, 4, 2, 2, TH)
        tw_core = np.stack([T[sl], w[sl]])
        in_maps.append({"x": x_core, "tw": tw_core})

    kwargs = {}
    if trace:
        kwargs["trace"] = True
        if trace_kwargs:
            kwargs["trace_kwargs"] = trace_kwargs
        if trace_cores is not None:
            kwargs["trace_cores"] = trace_cores
    res = run_bass_kernel_spmd(_get_nc(), in_maps, core_ids=list(range(N_CORES)),
                               **kwargs)
    per_sample = np.concatenate(
        [res.results[i]["out"][:, 0] for i in range(N_CORES)]
    )
    total = per_sample.astype(np.float64).sum() / (TH * B)
    return np.float32(LAMBDA_ANG * total), res


def kernel(mask_pred: np.ndarray, theta_min: np.ndarray,
           theta_max: np.ndarray) -> np.ndarray:
    loss, _ = _run(mask_pred, theta_min, theta_max)
    return np.asarray(loss, dtype=np.float32)
